# revision 10
# baseline (speedup 1.0000x reference)
"""Trainium2 Bass kernel for nn_CovarianceLayer.

Math (per image, PATCH=5):
    xc = center(x) - boxmean5x5(x)        # [1020, 1020]
    yc = center(y) - boxmean5x5(y)
    out = boxmean5x5(xc * yc)             # [1016, 1016]

Strategy:
  - Pure data parallel: 16 images -> 2 per NeuronCore across 8 cores.
  - Per image, process in 9 row-blocks of 128 input rows (120 output rows).
  - Horizontal pair-sums p[c] = a[c] + a[c+1] on DVE (1 pass per conv input).
  - Each 5x5 conv becomes 3 shifted banded matmuls on TensorE accumulating in
    PSUM: taps {0,1} and {3,4} come from the pair-sum at rhs offsets 0/3, tap
    {2} from the raw input at offset 2.  The vertical 5-tap band lives in the
    stationary [K, M] matrix; for the first conv the center-pixel delta is
    folded into the offset-2 band matrix, so PSUM holds xc/yc directly.
  - float32r matmuls (full fp32 storage, reduced-precision PE mode, 1 cyc/row).
  - Elementwise product xc*yc on GPSIMD, PSUM evacuations on ScalarE.
"""

import numpy as np

import concourse.bass as bass
import concourse.mybir as mybir
from concourse.tile import TileContext
from concourse.bass_utils import run_bass_kernel_spmd

PATCH = 5
H = W = 1024
ZW = W - 4          # 1020: width after first conv
OW = W - 8          # 1016: final output width
N_CORES = 8
B_TOTAL = 16
B_PER = B_TOTAL // N_CORES   # 2 images per core

R_OUT = 120         # output rows per block
XR = 128            # input rows loaded per block
ZR = R_OUT + 4      # 124 intermediate (xc/yc/z) rows per block
# block start rows (in output space); last block is shifted up so every block
# loads a full 128 input rows -- its first 64 output rows are recomputed and
# only rows [64:120) are stored.
BLOCK_STARTS = [0, 120, 240, 360, 480, 600, 720, 840, 896]

PS_C1_BUFS = 4
PS_OUT_BUFS = 4

F32 = mybir.dt.float32
F32R = mybir.dt.float32r


def _band(nrows, ncols, val):
    """W[k, m] = val for m <= k <= m+4 (vertical 5-tap band), else 0."""
    w = np.zeros((nrows, ncols), np.float32)
    for m in range(ncols):
        w[m:m + PATCH, m] = val
    return w


def _build_weights(inv_area):
    wm = _band(128, 128, -inv_area)            # -boxsum band
    wc = wm.copy()
    for m in range(128 - 2):
        wc[m + 2, m] += 1.0                    # + center-pixel delta
    wp = _band(128, 128, inv_area)             # +boxsum band (final conv)
    return wm, wc, wp


def _split_matmul_waits(nc):
    """Several walrus instruction structs (fused LDWEIGHTS+MATMUL for 4-byte
    dtypes, PSEUDO_DMA_DIRECT2D, ...) carry only one semaphore wait, while
    Tile freely attaches several.  Peel all but one wait off every
    instruction onto same-engine NoOps inserted just before it (same engine
    queue, so ordering semantics are identical)."""
    n = 0
    for f in nc.m.functions:
        for bb in f.blocks:
            i = 0
            while i < len(bb.instructions):
                inst = bb.instructions[i]
                si = inst.sync_info
                if (si is not None and len(si.on_wait) > 1
                        and not isinstance(inst, mybir.InstNoOp)):
                    extra = list(si.on_wait[:-1])
                    si.on_wait = [si.on_wait[-1]]
                    for w in extra:
                        nop = mybir.InstNoOp(name=f"I-mmwait-{n}", ins=[],
                                             outs=[])
                        n += 1
                        nop.engine = inst.engine
                        nop.sync_info = mybir.SyncInfo(on_wait=[w],
                                                       on_update=[])
                        nc.register_instruction(nop)
                        bb.instructions.insert(i, nop)
                        i += 1
                i += 1


def _build_nc():
    nc = bass.Bass()
    x_d = nc.dram_tensor("x", [B_PER, H, W], F32R, kind="ExternalInput")
    y_d = nc.dram_tensor("y", [B_PER, H, W], F32R, kind="ExternalInput")
    wm_d = nc.dram_tensor("wm", [128, 128], F32R, kind="ExternalInput")
    wc_d = nc.dram_tensor("wc", [128, 128], F32R, kind="ExternalInput")
    wp_d = nc.dram_tensor("wp", [128, 128], F32R, kind="ExternalInput")
    out_d = nc.dram_tensor("out", [B_PER, OW, OW], F32, kind="ExternalOutput")

    with TileContext(nc) as tc:
        with (
            tc.tile_pool(name="consts", bufs=1) as cpool,
            tc.tile_pool(name="io", bufs=4) as iopool,
            tc.tile_pool(name="work", bufs=3) as wpool,
            tc.tile_pool(name="ps_c1", bufs=PS_C1_BUFS, space="PSUM") as ps_c1,
            tc.tile_pool(name="ps_out", bufs=PS_OUT_BUFS, space="PSUM") as ps_out,
        ):
            wm_t = cpool.tile([128, 128], F32R)
            wc_t = cpool.tile([128, 128], F32R)
            wp_t = cpool.tile([128, 128], F32R)
            nc.sync.dma_start(out=wm_t[:, :], in_=wm_d[:, :])
            nc.sync.dma_start(out=wc_t[:, :], in_=wc_d[:, :])
            nc.sync.dma_start(out=wp_t[:, :], in_=wp_d[:, :])

            for b in range(B_PER):
                for s in BLOCK_STARTS:
                    xt = iopool.tile([XR, W], F32R, tag="xt")
                    yt = iopool.tile([XR, W], F32R, tag="yt")
                    nc.gpsimd.dma_start(out=xt[:, :], in_=x_d[b, s:s + XR, :])
                    nc.sync.dma_start(out=yt[:, :], in_=y_d[b, s:s + XR, :])

                    # horizontal pair sums  p[c] = a[c] + a[c+1]
                    px = wpool.tile([XR, W - 1], F32R, tag="px")
                    py = wpool.tile([XR, W - 1], F32R, tag="py")
                    nc.vector.tensor_add(out=px[:, :], in0=xt[:, 0:W - 1].bitcast(F32),
                                         in1=xt[:, 1:W].bitcast(F32))
                    nc.vector.tensor_add(out=py[:, :], in0=yt[:, 0:W - 1].bitcast(F32),
                                         in1=yt[:, 1:W].bitcast(F32))

                    # first conv: xc/yc = center - boxmean, via 3 banded
                    # matmuls per 512-column PSUM bank
                    xc_sb = wpool.tile([ZR, ZW], F32, tag="xc")
                    yc_sb = wpool.tile([ZR, ZW], F32, tag="yc")
                    for pt, raw, dst in ((px, xt, xc_sb), (py, yt, yc_sb)):
                        for c0, c1 in ((0, 512), (512, ZW)):
                            cps = ps_c1.tile([ZR, 512], F32, tag="c1")
                            n = c1 - c0
                            nc.tensor.matmul(
                                cps[:, :n],
                                wm_t[:XR, :ZR],
                                pt[:, c0:c1],
                                start=True, stop=False)
                            nc.tensor.matmul(
                                cps[:, :n],
                                wm_t[:XR, :ZR],
                                pt[:, 3 + c0:3 + c1],
                                start=False, stop=False)
                            nc.tensor.matmul(
                                cps[:, :n],
                                wc_t[:XR, :ZR],
                                raw[:, 2 + c0:2 + c1],
                                start=False, stop=True)
                            nc.scalar.copy(out=dst[:, c0:c1], in_=cps[:, :n])

                    # elementwise covariance term
                    z = wpool.tile([ZR, ZW], F32R, tag="z")
                    nc.gpsimd.tensor_mul(out=z[:, :], in0=xc_sb[:, :],
                                         in1=yc_sb[:, :])
                    pz = wpool.tile([ZR, ZW - 1], F32R, tag="pz")
                    nc.vector.tensor_add(out=pz[:, :], in0=z[:, 0:ZW - 1].bitcast(F32),
                                         in1=z[:, 1:ZW].bitcast(F32))

                    # final conv: out = boxmean(z)
                    out_sb = wpool.tile([R_OUT, OW], F32, tag="out_sb")
                    for c0, c1 in ((0, 512), (512, OW)):
                        ops = ps_out.tile([R_OUT, 512], F32, tag="po")
                        n = c1 - c0
                        nc.tensor.matmul(
                            ops[:, :n],
                            wp_t[:ZR, :R_OUT],
                            pz[:, c0:c1],
                            start=True, stop=False)
                        nc.tensor.matmul(
                            ops[:, :n],
                            wp_t[:ZR, :R_OUT],
                            pz[:, 2 + c0:2 + c1],
                            start=False, stop=False)
                        nc.tensor.matmul(
                            ops[:, :n],
                            wp_t[:ZR, :R_OUT],
                            z[:, 4 + c0:4 + c1],
                            start=False, stop=True)
                        nc.scalar.copy(out=out_sb[:, c0:c1], in_=ops[:, :n])

                    if s == BLOCK_STARTS[-1]:
                        # overlapped tail block: only store the fresh rows
                        skip = BLOCK_STARTS[-2] + R_OUT - s   # 64
                        nc.sync.dma_start(out=out_d[b, s + skip:s + R_OUT, :],
                                          in_=out_sb[skip:, :])
                    else:
                        nc.sync.dma_start(out=out_d[b, s:s + R_OUT, :],
                                          in_=out_sb[:, :])
    _split_matmul_waits(nc)
    return nc


def kernel(x, y, mean_mask, ones_mask):
    x = np.ascontiguousarray(np.asarray(x, np.float32).reshape(B_TOTAL, H, W))
    y = np.ascontiguousarray(np.asarray(y, np.float32).reshape(B_TOTAL, H, W))
    inv_area = float(np.asarray(mean_mask).reshape(-1)[0])   # 1/25
    wm, wc, wp = _build_weights(inv_area)

    nc = _build_nc()
    in_maps = []
    for c in range(N_CORES):
        in_maps.append({
            "x": np.ascontiguousarray(x[c * B_PER:(c + 1) * B_PER]),
            "y": np.ascontiguousarray(y[c * B_PER:(c + 1) * B_PER]),
            "wm": wm, "wc": wc, "wp": wp,
        })
    res = run_bass_kernel_spmd(nc, in_maps, list(range(N_CORES)))
    out = np.concatenate([r["out"] for r in res.results], axis=0)
    return out.reshape(B_TOTAL, 1, OW, OW).astype(np.float32)
